# revision 21
# baseline (speedup 1.0000x reference)
"""Differential attention (B=2, N=2048, D=1024, H=8, HEAD_DIM=128) on 8 trn2
NeuronCores. Head-parallel: core h computes head h end-to-end. The
heads->tokens reshard is pipelined: after each 512-token block tb finishes its
attention epilogue, a per-block AllToAll exchanges it (dest-sliced into
64-token chunks), and the output projection for a block PAIR (FD=128, two
one-bank column passes) runs interleaved with later blocks' attention. Each
core emits the token set {tb*512 + c*64 + i} (c = core).

Single PSUM pool scope for the whole program (s12 2x2 banks + U 3 banks + 1
rotating bank) - no inter-phase drain barriers. Startup is k-first: batch-0
k for all 4 half-blocks, then q/v for half-block 0, then the 8-block
attention loop starts with every remaining qkv group and output-projection
pass placed as a filler inside the PE slack of the exp-bound loop.

q/k are bf16 (fp32r streamed the PE at half rate and poisoned FWL). The two
score streams run row-packed (row_grp h0/h64). The epilogue uses the RMSNorm
scale-invariance od = d2*U1 - lam*d1*U2 (no reciprocals) and fused
scalar_tensor_tensor / tensor_tensor_reduce DVE ops.

DMA queue assignment keeps blocking chains off the PE/ACT-feeding queues:
epilogue transposes + A2A staging + aa gathers ride the idle GpSimd queue,
y stores the Vector queue, x loads the Sync queue. A dummy collective at
program start absorbs the ~40us CC cold-start; a dummy exp preloads the ACT
table set.
"""

import numpy as np

import concourse.bass as bass
import concourse.mybir as mybir
import concourse.tile as tile
from concourse.bass_utils import run_bass_kernel_spmd
from concourse.vector_clock import ScopedClock

# ---------------------------------------------------------------- constants
B, N, D = 2, 2048, 1024
H, HD = 8, 128
DQK = HD // 2
PROJ = H * HD
T = B * N  # 4096 flattened tokens
NCORES = 8
TBLK = T // NCORES  # 512 tokens per core for the output projection
LAMBDA_INIT = 0.8 - 0.6 * float(np.exp(-0.3 * 12))
SCALE = DQK ** -0.5
EPS = 1e-6

KB = N // 128  # 16 key chunks per batch
QB = N // 512  # 4 query blocks of 512 per batch
NBLK = B * QB  # 8 global 512-token blocks == NCORES
DC = D // 128  # contraction chunks for the qkv projection

FP = mybir.dt.float32
BF = mybir.dt.bfloat16

# (stream, sub) -> column offset of the 129-wide accumulator region inside the
# 3-bank U tile [128, 1536]; regions never cross a 512-col PSUM bank boundary.
UREG = [0, 129, 258, 512, 641, 770, 1024, 1153]  # index = s * 4 + sub
UFIRST = ((0, 0), (0, 3), (1, 2))  # (s, sub) pairs that first touch a U bank


# ------------------------------------------------- walrus drain workaround
# This container's walrus rejects instructions carrying >1 sync wait.
def _split_waits(nc, inst, max_waits=1):
    si = inst.ins.sync_info
    if si is None:
        return
    waits = list(si.on_wait)
    if len(waits) <= max_waits:
        return
    si.on_wait = waits[:max_waits]
    for w in waits[max_waits:]:
        d2 = nc.sync.drain(fusable=False)
        si2 = d2.ins.sync_info
        if si2 is None:
            d2.ins.sync_info = mybir.SyncInfo(on_wait=[w], on_update=[])
        else:
            si2.on_wait = [w]


def _split_all_multiwaits(nc, max_waits=1):
    """Hoist extra sync-waits onto fresh NoOps inserted just before the
    instruction on the same engine (engines dispatch in order)."""
    uid = 0
    for fn in nc.m.functions:
        for bb in fn.blocks:
            il = bb.instructions
            changed = False
            out = []
            for inst in il:
                si = inst.sync_info
                waits = list(si.on_wait) if si is not None else []
                if len(waits) > max_waits:
                    for w in waits[:-max_waits]:
                        ev = mybir.InstEventSemaphore(
                            name=f"waitsplit_{uid}",
                            sync_info=mybir.SyncInfo(on_wait=[w], on_update=[]),
                            engine=inst.engine,
                        )
                        uid += 1
                        out.append(ev)
                    si.on_wait = waits[-max_waits:]
                    if inst.sync_info is not si:
                        inst.sync_info = si
                    changed = True
                out.append(inst)
            if changed:
                bb.instructions = out


def _patched_drain_and_barrier(self, tick_clock, wait_clock):
    nc = self.nc
    drain_inst = nc.sync.drain(fusable=False)
    wait_clock.add_sem_waits(
        drain_inst.ins, ScopedClock({None: tick_clock.global_clock})
    )
    _split_waits(nc, drain_inst)
    nc.all_engine_barrier()
    assert self.sems is not None
    popped = nc._tile_sem_poison_stack.pop()
    assert popped is self._sem_poison
    nc.clear_and_free_semaphores(list(self.sems.allocated().values()))
    nc.all_engine_barrier()


tile.TileContext._drain_and_barrier = _patched_drain_and_barrier


# ---------------------------------------------------------------- program
def build_program(dbg=False, reps=1, skip_cc=False):
    nc = bass.Bass(
        "TRN2",
        target_bir_lowering=False,
        debug=False,
        enable_asserts=True,
        num_devices=NCORES,
    )

    xT = nc.dram_tensor("xT", [D, T], BF, kind="ExternalInput")
    wq = nc.dram_tensor("wq", [128, DC * HD], BF, kind="ExternalInput")
    wk = nc.dram_tensor("wk", [128, DC * HD], BF, kind="ExternalInput")
    wv = nc.dram_tensor("wv", [128, DC * HD], BF, kind="ExternalInput")
    wp = nc.dram_tensor("wp", [128, H * D], BF, kind="ExternalInput")
    lam = nc.dram_tensor("lam", [128, 1], FP, kind="ExternalInput")
    yT = nc.dram_tensor("yT", [D, TBLK], FP, kind="ExternalOutput")
    if dbg:
        d_in = nc.dram_tensor("d_in", [NBLK, NCORES, 128, 64], BF,
                              kind="ExternalOutput")
        d_out = nc.dram_tensor("d_out", [NBLK, NCORES, 128, 64], BF,
                               kind="ExternalOutput")
        d_q = nc.dram_tensor("d_q", [B, 128, N], BF, kind="ExternalOutput")
        d_k = nc.dram_tensor("d_k", [B, 128, N], BF, kind="ExternalOutput")
        d_v = nc.dram_tensor("d_v", [B, 128, KB * (HD + 1)], BF,
                             kind="ExternalOutput")
        d_usb = nc.dram_tensor("d_usb", [128, 1536], FP, kind="ExternalOutput")
        d_od = nc.dram_tensor("d_od", [4, 128, 128], FP, kind="ExternalOutput")
        d_rs = nc.dram_tensor("d_rs", [128, 8], FP, kind="ExternalOutput")

    with tile.TileContext(nc, num_cores=NCORES) as tc:
        with (
            tc.tile_pool(name="consts", bufs=1) as consts,
            tc.tile_pool(name="xa", bufs=3) as xa,
            tc.tile_pool(name="pp", bufs=4) as pp,
            tc.tile_pool(name="se", bufs=2) as se,
            tc.tile_pool(name="so", bufs=4) as so,
            tc.tile_pool(name="sc", bufs=2) as sc,
            tc.tile_pool(name="dram", bufs=1, space="DRAM") as dram,
            tc.tile_pool(name="ps", bufs=1, space="PSUM") as ps,
            tc.tile_pool(name="pu", bufs=1, space="PSUM") as pu,
            tc.tile_pool(name="px", bufs=1, space="PSUM") as px,
        ):
            lam_sb = consts.tile([128, 1], FP)
            nc.sync.dma_start(lam_sb[:], lam[:])
            wq_sb = consts.tile([128, DC, HD], BF)
            wk_sb = consts.tile([128, DC, HD], BF)
            wv_sb = consts.tile([128, DC, HD], BF)
            nc.scalar.dma_start(wk_sb[:], wk.rearrange("p (c m) -> p c m", c=DC))

            # warm the ACT table set (exp/ln) off the critical path
            warm = consts.tile([128, 1], FP)
            nc.scalar.activation(warm[:], lam_sb[:],
                                 mybir.ActivationFunctionType.Exp)

            # warm the collective stack: small aligned AllToAll (1KB/chunk)
            # absorbs the ~40us CC init off the critical path
            cc_warm_in = dram.tile([NCORES, 512], BF)
            cc_warm_out = dram.tile([NCORES, 512], BF)
            warm8 = consts.tile([8, 512], BF)
            nc.vector.memset(warm8[:], 0.0)
            eps_sb = consts.tile([128, 1], FP)
            nc.vector.memset(eps_sb[:], EPS)
            nc.gpsimd.dma_start(cc_warm_in[:], warm8[:])
            if not skip_cc:
                nc.gpsimd.collective_compute(
                    "AllToAll",
                    mybir.AluOpType.bypass,
                    replica_groups=[list(range(NCORES))],
                    ins=[cc_warm_in.opt()],
                    outs=[cc_warm_out.opt()],
                )

            wp_sb = consts.tile([128, H, D], BF)

            qT_b = [consts.tile([128, N], BF, name=f"qT_{b}") for b in range(B)]
            kT_b = [consts.tile([128, N], BF, name=f"kT_{b}") for b in range(B)]
            # v per (batch, key-chunk): [key, head_dim] plus a ones column
            # (col 128) so the PV matmul also accumulates the softmax denom.
            va_b = [consts.tile([128, KB, HD + 1], BF, name=f"va_{b}") for b in range(B)]
            for b in range(B):
                nc.vector.memset(va_b[b][:, :, HD : HD + 1], 1.0)

            # pipelined reshard buffers, one collective per 512-token block:
            # [block, dest-core, feature, 64 tokens]
            a2a_in = dram.tile([NBLK, NCORES, 128, 64], BF)
            a2a_out = dram.tile([NBLK, NCORES, 128, 64], BF)

            xT_view = xT.rearrange("(c p) t -> p c t", p=128)

            xxs = {}

            def load_x(tp, engines):
                """DMA the 8 contraction chunks for 1024-token pair tp."""
                ts2 = slice(tp * 1024, (tp + 1) * 1024)
                xx = []
                for c in range(DC):
                    t = xa.tile([128, 1024], BF, tag=f"xx{c}", name=f"xx_{tp}_{c}")
                    engines[c % len(engines)].dma_start(t[:], xT_view[:, c, ts2])
                    xx.append(t)
                xxs[tp] = xx

            def proj_group(which, b, j, pst):
                """One q/k projection group for half-block j of batch b into
                the [128, 512] PSUM region pst, then copy out feature-major."""
                xx = xxs[2 * b + j // 2]
                hs = slice((j % 2) * 512, (j % 2 + 1) * 512)
                bs = slice(j * 512, (j + 1) * 512)
                w_sb = wq_sb if which == "q" else wk_sb
                dst = (qT_b if which == "q" else kT_b)[b]
                for c in range(DC):
                    nc.tensor.matmul(pst, w_sb[:, c, :], xx[c][:, hs],
                                     start=(c == 0), stop=(c == DC - 1))
                nc.vector.tensor_copy(dst[:, bs], pst)

            def v_group(b, j, pxt):
                """v for half-block j of batch b, directly token-major."""
                xx = xxs[2 * b + j // 2]
                vv = pxt.rearrange("p (s m) -> p s m", s=4)
                for sub in range(4):
                    ss = slice((j % 2) * 512 + sub * 128,
                               (j % 2) * 512 + (sub + 1) * 128)
                    for c in range(DC):
                        nc.tensor.matmul(
                            vv[:, sub, :], xxs[2 * b + j // 2][c][:, ss],
                            wv_sb[:, c, :],
                            start=(c == 0 and sub == 0),
                            stop=(c == DC - 1),
                        )
                kb0 = j * 4
                nc.vector.tensor_copy(va_b[b][:, kb0 : kb0 + 4, 0:HD], vv[:])

            def make_qk_filler(which, b, j):
                def f():
                    pst = px.tile([128, 512], FP, tag="px",
                                  name=f"px_{which}_{b}_{j}")
                    proj_group(which, b, j, pst[:])
                return f

            def make_v_filler(b, j):
                def f():
                    pxt = px.tile([128, 512], FP, tag="px", name=f"px_v_{b}_{j}")
                    v_group(b, j, pxt[:])
                return f

            def score_exp(b, qb, kb):
                """scores + exp for one 128-key chunk of block (b, qb)."""
                qs = slice(qb * 512, (qb + 1) * 512)
                ks = slice(kb * 128, (kb + 1) * 128)
                s12 = ps.tile([128, 1024], FP, tag="s12", bufs=2)
                nc.tensor.matmul(s12[:, 0:512], kT_b[b][0:64, ks],
                                 qT_b[b][0:64, qs], start=True, stop=True)
                nc.tensor.matmul(s12[:, 512:1024], kT_b[b][64:128, ks],
                                 qT_b[b][64:128, qs], start=True, stop=True)
                p12 = pp.tile([128, 1024], BF, tag="p12")
                nc.scalar.activation(p12[:], s12[:],
                                     mybir.ActivationFunctionType.Exp)
                return p12

            def pv(b, p12, U, kb):
                vak = va_b[b][:, kb, :]
                for s in range(2):
                    for sub in range(4):
                        r = UREG[s * 4 + sub]
                        nc.tensor.matmul(
                            U[:, r : r + HD + 1],
                            p12[:, s * 512 + sub * 128 : s * 512 + (sub + 1) * 128],
                            vak,
                            start=(kb == 0 and (s, sub) in UFIRST),
                            stop=(kb == KB - 1),
                        )

            def epilogue_part1(tb, U):
                """Differential combine without softmax division: od =
                d2*U1 - lam*d1*U2 = d1*d2*(a1 - lam*a2); RMSNorm is scale-
                invariant so the 1/(d1*d2) factor drops out (eps folded as a
                negligible Ln bias). Fused sum-of-squares via ttr."""
                Usb = se.tile([128, 1536], FP, tag="usb", name=f"usb_{tb}")
                usb_tiles[tb] = Usb
                nc.vector.tensor_copy(Usb[:], U[:])
                ssum4 = se.tile([128, 4], FP, tag="ms")
                od_t = [None] * 4
                for sub in range(4):
                    r1 = UREG[sub]
                    r2 = UREG[4 + sub]
                    d1 = Usb[:, r1 + HD : r1 + HD + 1]
                    d2 = Usb[:, r2 + HD : r2 + HD + 1]
                    w2 = se.tile([128, 1], FP, tag=f"w2_{sub}")
                    nc.vector.tensor_mul(w2[:], d1, lam_sb[:])
                    t2 = se.tile([128, 128], FP, tag=f"t2_{sub}")
                    nc.vector.tensor_scalar_mul(t2[:], Usb[:, r2 : r2 + HD], w2[:])
                    od = se.tile([128, 128], FP, tag=f"od_{sub}")
                    nc.vector.scalar_tensor_tensor(
                        od[:], Usb[:, r1 : r1 + HD], d2, t2[:],
                        mybir.AluOpType.mult, mybir.AluOpType.subtract,
                    )
                    od_t[sub] = od
                    sq = se.tile([128, 128], FP, tag=f"sq_{sub}")
                    nc.vector.scalar_tensor_tensor(
                        sq[:], od[:], 1.0, od[:],
                        mybir.AluOpType.mult, mybir.AluOpType.mult,
                        accum_out=ssum4[:, sub : sub + 1],
                    )
                return (tb, od_t, ssum4)

            def epilogue_part2(tb, od_t, ssum4):
                """RMSNorm scale (Ln/Exp share the attention exps' ACT table
                set), transpose, stage, fire this block's collective. The
                transposes/staging ride the GpSimd queue so a stall never
                blocks PE- or ACT-feeding DMAs. Deferred into the next
                block's chunk loop so the ACT queue never stalls."""
                rt = se.tile([128, 4], FP, tag="rt")
                nc.scalar.activation(rt[:], ssum4[:],
                                     mybir.ActivationFunctionType.Ln,
                                     scale=1.0 / HD, bias=eps_sb[:])
                rs = se.tile([128, 4], FP, tag="rs")
                nc.scalar.activation(rs[:], rt[:],
                                     mybir.ActivationFunctionType.Exp, scale=-0.5)
                onT4 = so.tile([128, 4, 128], BF, tag="onT4")
                for sub in range(4):
                    on = se.tile([128, 128], BF, tag=f"on_{sub}")
                    nc.vector.tensor_scalar_mul(
                        on[:], od_t[sub][:], rs[:, sub : sub + 1]
                    )
                    nc.sync.dma_start_transpose(onT4[:, sub, :], on[:])
                for sub in range(4):
                    nc.sync.dma_start(
                        a2a_in[tb, 2 * sub : 2 * sub + 2].rearrange(
                            "c p t -> p c t"),
                        onT4[:, sub, :].rearrange("p (c t) -> p c t", c=2),
                    )
                if skip_cc:
                    nc.gpsimd.dma_start(a2a_out[tb], a2a_in[tb])
                else:
                    nc.gpsimd.collective_compute(
                        "AllToAll",
                        mybir.AluOpType.bypass,
                        replica_groups=[list(range(NCORES))],
                        ins=[a2a_in[tb].opt()],
                        outs=[a2a_out[tb].opt()],
                    )

            def epilogue_tail(tb, U):
                """Last block's epilogue: per-sub pipelined DVE->ACT->DMA
                chain reading U PSUM directly (no Usb copy - U is not reused),
                transposes on the now-idle ACT queue, stores on sync, so the
                final A2A fires ~5us after the last PV matmul."""
                onT4 = so.tile([128, 4, 128], BF, tag="onT4")
                for sub in range(4):
                    r1 = UREG[sub]
                    r2 = UREG[4 + sub]
                    d1 = U[:, r1 + HD : r1 + HD + 1]
                    d2 = U[:, r2 + HD : r2 + HD + 1]
                    w2 = se.tile([128, 1], FP, tag=f"w2_{sub}")
                    nc.vector.tensor_mul(w2[:], d1, lam_sb[:])
                    t2 = se.tile([128, 128], FP, tag=f"t2_{sub}")
                    nc.vector.tensor_scalar_mul(t2[:], U[:, r2 : r2 + HD], w2[:])
                    od = se.tile([128, 128], FP, tag=f"od_{sub}")
                    nc.vector.scalar_tensor_tensor(
                        od[:], U[:, r1 : r1 + HD], d2, t2[:],
                        mybir.AluOpType.mult, mybir.AluOpType.subtract,
                    )
                    sq = se.tile([128, 128], FP, tag=f"sq_{sub}")
                    ss1 = se.tile([128, 1], FP, tag=f"ss_{sub}")
                    nc.vector.scalar_tensor_tensor(
                        sq[:], od[:], 1.0, od[:],
                        mybir.AluOpType.mult, mybir.AluOpType.mult,
                        accum_out=ss1[:],
                    )
                    rt1 = se.tile([128, 1], FP, tag=f"rt_{sub}")
                    nc.scalar.activation(rt1[:], ss1[:],
                                         mybir.ActivationFunctionType.Ln,
                                         scale=1.0 / HD, bias=eps_sb[:])
                    rs1 = se.tile([128, 1], FP, tag=f"rs_{sub}")
                    nc.scalar.activation(rs1[:], rt1[:],
                                         mybir.ActivationFunctionType.Exp,
                                         scale=-0.5)
                    on = se.tile([128, 128], BF, tag=f"on_{sub}")
                    nc.vector.tensor_scalar_mul(on[:], od[:], rs1[:])
                    nc.scalar.dma_start_transpose(onT4[:, sub, :], on[:])
                    nc.sync.dma_start(
                        a2a_in[tb, 2 * sub : 2 * sub + 2].rearrange(
                            "c p t -> p c t"),
                        onT4[:, sub, :].rearrange("p (c t) -> p c t", c=2),
                    )
                nc.gpsimd.collective_compute(
                    "AllToAll",
                    mybir.AluOpType.bypass,
                    replica_groups=[list(range(NCORES))],
                    ins=[a2a_in[tb].opt()],
                    outs=[a2a_out[tb].opt()],
                )

            aa_tiles = {}
            aa_loaded = set()
            usb_tiles = {}

            def load_aa(pair, tix):
                """Gather all 8 heads of block 2*pair+tix (one 3D-AP DMA)."""
                if pair not in aa_tiles:
                    aa_tiles[pair] = sc.tile([128, H, 2, 64], BF, tag="aa",
                                             name=f"aa_{pair}")
                nc.sync.dma_start(
                    aa_tiles[pair][:, :, tix],
                    a2a_out[2 * pair + tix].rearrange("h p t -> p h t"),
                )
                aa_loaded.add((pair, tix))

            def cproj_pass(pair, pi):
                """Output projection for block pair (2*pair, 2*pair+1): this
                core's 128 tokens; column pass pi covers output chunks
                oc = 4*pi .. 4*pi+3 (one PSUM bank)."""
                for tix in range(2):
                    if (pair, tix) not in aa_loaded:
                        load_aa(pair, tix)
                aa = aa_tiles[pair]
                ypst = px.tile([128, 512], FP, tag="px", name=f"yps_{pair}_{pi}")
                yps = ypst[:].rearrange("p (o t) -> p o t", o=4)
                for oci in range(4):
                    oc = 4 * pi + oci
                    for hh in range(H):
                        nc.tensor.matmul(
                            yps[:, oci, :],
                            wp_sb[:, hh, oc * 128 : (oc + 1) * 128],
                            aa[:, hh].rearrange("p b t -> p (b t)"),
                            start=(hh == 0),
                            stop=(hh == H - 1),
                        )
                yo = sc.tile([128, 512], FP, tag="yo", name=f"yo_{pair}_{pi}")
                nc.vector.tensor_copy(yo[:], ypst[:])
                nc.gpsimd.dma_start(
                    yT.rearrange("(oc p) t -> p oc t", p=128)[
                        :, 4 * pi : 4 * pi + 4, pair * 128 : (pair + 1) * 128
                    ],
                    yo[:].rearrange("p (o t) -> p o t", o=4),
                )

            def b_block(b, qb, tb, fillers, pending, last=False):
                """One 512-token attention block, software-pipelined: scores
                and exp lead the PV accumulation by one chunk; the previous
                block's deferred epilogue tail lands at kb==5; `fillers`
                maps kb -> callable (qkv groups or output-proj passes)."""
                U = pu.tile([128, 1536], FP, tag="U", name=f"U_{tb}")
                p12_prev = None
                for kb in range(KB + 1):
                    p12 = score_exp(b, qb, kb) if kb < KB else None
                    if kb >= 1:
                        pv(b, p12_prev, U[:], kb - 1)
                    if p12 is not None:
                        p12_prev = p12
                    if kb == 5 and pending is not None:
                        epilogue_part2(*pending)
                        pending = None
                    f = fillers.get(kb)
                    if f is not None:
                        f()
                if last:
                    return ("tail", tb, U)
                return epilogue_part1(tb, U[:])

            # ============ startup: x DMA + batch-0 k, then q/v of hb 0 =====
            load_x(0, [nc.sync, nc.scalar])
            nc.sync.dma_start(wq_sb[:], wq.rearrange("p (c m) -> p c m", c=DC))
            load_x(1, [nc.sync, nc.scalar])
            nc.scalar.dma_start(wv_sb[:], wv.rearrange("p (c m) -> p c m", c=DC))
            nc.scalar.dma_start(wp_sb[:], wp.rearrange("p (h m) -> p h m", h=H))

            sA = ps.tile([128, 1024], FP, tag="s12", bufs=2, name="ph1_k0q0")
            proj_group("k", 0, 0, sA[:, 0:512])
            proj_group("q", 0, 0, sA[:, 512:1024])

            # ============ the 8-block attention loop =====================
            def seq(*fs):
                def g():
                    for f in fs:
                        f()
                return g

            def make_aa_filler(pair):
                return lambda: (load_aa(pair, 0), load_aa(pair, 1))

            filler_map = {
                0: {0: make_v_filler(0, 0), 1: make_qk_filler("k", 0, 1),
                    2: make_v_filler(0, 1), 5: make_qk_filler("k", 0, 2),
                    6: make_v_filler(0, 2), 9: make_qk_filler("k", 0, 3),
                    10: make_v_filler(0, 3), 13: make_qk_filler("q", 0, 1)},
                1: {2: make_qk_filler("q", 0, 2), 7: make_qk_filler("q", 0, 3),
                    12: make_qk_filler("k", 1, 0)},
                2: {2: make_qk_filler("k", 1, 1), 7: make_qk_filler("k", 1, 2),
                    12: make_qk_filler("k", 1, 3)},
                3: {2: make_qk_filler("q", 1, 0), 7: make_v_filler(1, 0),
                    12: make_v_filler(1, 1)},
                4: {2: make_v_filler(1, 2), 6: make_v_filler(1, 3),
                    10: make_qk_filler("q", 1, 1)},
                5: {2: seq(make_aa_filler(0), make_qk_filler("q", 1, 2)),
                    6: (lambda: cproj_pass(0, 0)),
                    13: (lambda: cproj_pass(0, 1))},
                6: {2: seq(make_aa_filler(1), make_qk_filler("q", 1, 3)),
                    7: (lambda: cproj_pass(1, 0)),
                    13: (lambda: cproj_pass(1, 1))},
                7: {2: make_aa_filler(2),
                    6: (lambda: cproj_pass(2, 0)),
                    13: (lambda: cproj_pass(2, 1))},
            }

            pending = None
            for tb in range(NBLK):
                if tb == 1:
                    load_x(2, [nc.sync])
                elif tb == 2:
                    load_x(3, [nc.sync])
                b, qb = tb // QB, tb % QB
                pending = b_block(b, qb, tb, filler_map.get(tb, {}), pending,
                                  last=(tb == NBLK - 1))

            # ============ tail: last block's pipelined epilogue + pair 3 ==
            _, tb7, U7 = pending
            epilogue_tail(tb7, U7[:])
            load_aa(3, 0)
            cproj_pass(3, 0)
            cproj_pass(3, 1)

            if dbg:
                # round-trip through SBUF at the very end so the dump
                # reflects post-collective DRAM state
                for tb in range(NBLK):
                    for hh in range(NCORES):
                        dt_ = sc.tile([128, 64], BF, tag="dbg")
                        nc.sync.dma_start(dt_[:], a2a_in[tb, hh])
                        nc.sync.dma_start(d_in[tb, hh], dt_[:])
                        dt2 = sc.tile([128, 64], BF, tag="dbg2")
                        nc.sync.dma_start(dt2[:], a2a_out[tb, hh])
                        nc.sync.dma_start(d_out[tb, hh], dt2[:])
                for b in range(B):
                    dq = sc.tile([128, N], BF, tag="dbgq")
                    nc.sync.dma_start(dq[:], qT_b[b][:])
                    nc.sync.dma_start(d_q[b], dq[:])
                    dk = sc.tile([128, N], BF, tag="dbgk")
                    nc.sync.dma_start(dk[:], kT_b[b][:])
                    nc.sync.dma_start(d_k[b], dk[:])
                    dv = sc.tile([128, KB * (HD + 1)], BF, tag="dbgv")
                    nc.sync.dma_start(
                        dv[:], va_b[b][:].rearrange("p a c -> p (a c)"))
                    nc.sync.dma_start(d_v[b], dv[:])

    _split_all_multiwaits(nc)
    return nc


_PROGRAM = None


def _get_program():
    global _PROGRAM
    if _PROGRAM is None:
        _PROGRAM = build_program()
    return _PROGRAM


# ---------------------------------------------------------------- host side
def _prep_in_maps(x, w_qkv, w_proj, lambda_q1, lambda_k1, lambda_q2, lambda_k2,
                  rms_weight):
    import ml_dtypes

    x = np.asarray(x, dtype=np.float32)
    w_qkv = np.asarray(w_qkv, dtype=np.float32)
    w_proj = np.asarray(w_proj, dtype=np.float32)
    xT = np.ascontiguousarray(x.reshape(T, D).T).astype(ml_dtypes.bfloat16)
    lam_val = (
        float(np.exp(np.sum(np.asarray(lambda_q1, np.float64) * np.asarray(lambda_k1, np.float64))))
        - float(np.exp(np.sum(np.asarray(lambda_q2, np.float64) * np.asarray(lambda_k2, np.float64))))
        + LAMBDA_INIT
    )
    lam_arr = np.full((128, 1), lam_val, dtype=np.float32)
    # fold rms_weight and (1 - lambda_init) into the output projection rows
    rw = np.asarray(rms_weight, np.float32)
    wp_full = np.ascontiguousarray(
        w_proj * np.tile(rw, H)[:, None] * np.float32(1.0 - LAMBDA_INIT)
    )

    def chunked(w):  # [D, HD] -> [128, DC*HD] with [p, c*HD+m] = w[c*128+p, m]
        dc = D // 128
        return np.ascontiguousarray(
            w.reshape(dc, 128, HD).transpose(1, 0, 2).reshape(128, dc * HD)
        ).astype(ml_dtypes.bfloat16)

    wp_dev = np.ascontiguousarray(
        wp_full.reshape(H, 128, D).transpose(1, 0, 2).reshape(128, H * D)
    ).astype(ml_dtypes.bfloat16)
    in_maps = []
    for h in range(NCORES):
        hs = slice(h * HD, (h + 1) * HD)
        in_maps.append(
            {
                "xT": xT,
                "wq": chunked(np.ascontiguousarray(w_qkv[:, hs]) * np.float32(SCALE)),
                "wk": chunked(w_qkv[:, PROJ + h * HD : PROJ + (h + 1) * HD]),
                "wv": chunked(w_qkv[:, 2 * PROJ + h * HD : 2 * PROJ + (h + 1) * HD]),
                "wp": wp_dev,
                "lam": lam_arr,
            }
        )
    return in_maps


def _assemble(results):
    y = np.empty((T, D), dtype=np.float32)
    for c in range(NCORES):
        yTc = results[c]["yT"]  # [D, 512], cols ordered (tb, i)
        for tb in range(NBLK):
            y[tb * 512 + c * 64 : tb * 512 + (c + 1) * 64, :] = (
                yTc[:, tb * 64 : (tb + 1) * 64].T
            )
    return y.reshape(B, N, D)


def kernel(x, w_qkv, w_proj, lambda_q1, lambda_k1, lambda_q2, lambda_k2,
           rms_weight):
    nc = _get_program()
    in_maps = _prep_in_maps(
        x, w_qkv, w_proj, lambda_q1, lambda_k1, lambda_q2, lambda_k2, rms_weight
    )
    res = run_bass_kernel_spmd(nc, in_maps, list(range(NCORES)))
    return _assemble(res.results)


# revision 22
# speedup vs baseline: 1.0106x; 1.0106x over previous
"""Differential attention (B=2, N=2048, D=1024, H=8, HEAD_DIM=128) on 8 trn2
NeuronCores. Head-parallel: core h computes head h end-to-end. The
heads->tokens reshard is pipelined: after each 512-token block tb finishes its
attention epilogue, a per-block AllToAll exchanges it (dest-sliced into
64-token chunks), and the output projection for a block PAIR (FD=128, two
one-bank column passes) runs interleaved with later blocks' attention. Each
core emits the token set {tb*512 + c*64 + i} (c = core).

Single PSUM pool scope for the whole program (s12 2x2 banks + U 3 banks + 1
rotating bank) - no inter-phase drain barriers. Startup is k-first: batch-0
k for all 4 half-blocks, then q/v for half-block 0, then the 8-block
attention loop starts with every remaining qkv group and output-projection
pass placed as a filler inside the PE slack of the exp-bound loop.

q/k are bf16 (fp32r streamed the PE at half rate and poisoned FWL). The two
score streams run row-packed (row_grp h0/h64). The epilogue uses the RMSNorm
scale-invariance od = d2*U1 - lam*d1*U2 (no reciprocals) and fused
scalar_tensor_tensor / tensor_tensor_reduce DVE ops.

DMA queue assignment keeps blocking chains off the PE/ACT-feeding queues:
epilogue transposes + A2A staging + aa gathers ride the idle GpSimd queue,
y stores the Vector queue, x loads the Sync queue. A dummy collective at
program start absorbs the ~40us CC cold-start; a dummy exp preloads the ACT
table set.
"""

import numpy as np

import concourse.bass as bass
import concourse.mybir as mybir
import concourse.tile as tile
from concourse.bass_utils import run_bass_kernel_spmd
from concourse.vector_clock import ScopedClock

# ---------------------------------------------------------------- constants
B, N, D = 2, 2048, 1024
H, HD = 8, 128
DQK = HD // 2
PROJ = H * HD
T = B * N  # 4096 flattened tokens
NCORES = 8
TBLK = T // NCORES  # 512 tokens per core for the output projection
LAMBDA_INIT = 0.8 - 0.6 * float(np.exp(-0.3 * 12))
SCALE = DQK ** -0.5
EPS = 1e-6

KB = N // 128  # 16 key chunks per batch
QB = N // 512  # 4 query blocks of 512 per batch
NBLK = B * QB  # 8 global 512-token blocks == NCORES
DC = D // 128  # contraction chunks for the qkv projection

FP = mybir.dt.float32
BF = mybir.dt.bfloat16

# (stream, sub) -> column offset of the 129-wide accumulator region inside the
# 3-bank U tile [128, 1536]; regions never cross a 512-col PSUM bank boundary.
UREG = [0, 129, 258, 512, 641, 770, 1024, 1153]  # index = s * 4 + sub
UFIRST = ((0, 0), (0, 3), (1, 2))  # (s, sub) pairs that first touch a U bank


# ------------------------------------------------- walrus drain workaround
# This container's walrus rejects instructions carrying >1 sync wait.
def _split_waits(nc, inst, max_waits=1):
    si = inst.ins.sync_info
    if si is None:
        return
    waits = list(si.on_wait)
    if len(waits) <= max_waits:
        return
    si.on_wait = waits[:max_waits]
    for w in waits[max_waits:]:
        d2 = nc.sync.drain(fusable=False)
        si2 = d2.ins.sync_info
        if si2 is None:
            d2.ins.sync_info = mybir.SyncInfo(on_wait=[w], on_update=[])
        else:
            si2.on_wait = [w]


def _split_all_multiwaits(nc, max_waits=1):
    """Hoist extra sync-waits onto fresh NoOps inserted just before the
    instruction on the same engine (engines dispatch in order)."""
    uid = 0
    for fn in nc.m.functions:
        for bb in fn.blocks:
            il = bb.instructions
            changed = False
            out = []
            for inst in il:
                si = inst.sync_info
                waits = list(si.on_wait) if si is not None else []
                if len(waits) > max_waits:
                    for w in waits[:-max_waits]:
                        ev = mybir.InstEventSemaphore(
                            name=f"waitsplit_{uid}",
                            sync_info=mybir.SyncInfo(on_wait=[w], on_update=[]),
                            engine=inst.engine,
                        )
                        uid += 1
                        out.append(ev)
                    si.on_wait = waits[-max_waits:]
                    if inst.sync_info is not si:
                        inst.sync_info = si
                    changed = True
                out.append(inst)
            if changed:
                bb.instructions = out


def _patched_drain_and_barrier(self, tick_clock, wait_clock):
    nc = self.nc
    drain_inst = nc.sync.drain(fusable=False)
    wait_clock.add_sem_waits(
        drain_inst.ins, ScopedClock({None: tick_clock.global_clock})
    )
    _split_waits(nc, drain_inst)
    nc.all_engine_barrier()
    assert self.sems is not None
    popped = nc._tile_sem_poison_stack.pop()
    assert popped is self._sem_poison
    nc.clear_and_free_semaphores(list(self.sems.allocated().values()))
    nc.all_engine_barrier()


tile.TileContext._drain_and_barrier = _patched_drain_and_barrier


# ---------------------------------------------------------------- program
def build_program(dbg=False, reps=1, skip_cc=False):
    nc = bass.Bass(
        "TRN2",
        target_bir_lowering=False,
        debug=False,
        enable_asserts=True,
        num_devices=NCORES,
    )

    xT = nc.dram_tensor("xT", [D, T], BF, kind="ExternalInput")
    wq = nc.dram_tensor("wq", [128, DC * HD], BF, kind="ExternalInput")
    wk = nc.dram_tensor("wk", [128, DC * HD], BF, kind="ExternalInput")
    wv = nc.dram_tensor("wv", [128, DC * HD], BF, kind="ExternalInput")
    wp = nc.dram_tensor("wp", [128, H * D], BF, kind="ExternalInput")
    lam = nc.dram_tensor("lam", [128, 1], FP, kind="ExternalInput")
    yT = nc.dram_tensor("yT", [D, TBLK], FP, kind="ExternalOutput")
    if dbg:
        d_in = nc.dram_tensor("d_in", [NBLK, NCORES, 128, 64], BF,
                              kind="ExternalOutput")
        d_out = nc.dram_tensor("d_out", [NBLK, NCORES, 128, 64], BF,
                               kind="ExternalOutput")
        d_q = nc.dram_tensor("d_q", [B, 128, N], BF, kind="ExternalOutput")
        d_k = nc.dram_tensor("d_k", [B, 128, N], BF, kind="ExternalOutput")
        d_v = nc.dram_tensor("d_v", [B, 128, KB * (HD + 1)], BF,
                             kind="ExternalOutput")
        d_usb = nc.dram_tensor("d_usb", [128, 1536], FP, kind="ExternalOutput")
        d_od = nc.dram_tensor("d_od", [4, 128, 128], FP, kind="ExternalOutput")
        d_rs = nc.dram_tensor("d_rs", [128, 8], FP, kind="ExternalOutput")

    with tile.TileContext(nc, num_cores=NCORES) as tc:
        with (
            tc.tile_pool(name="consts", bufs=1) as consts,
            tc.tile_pool(name="xa", bufs=3) as xa,
            tc.tile_pool(name="pp", bufs=4) as pp,
            tc.tile_pool(name="se", bufs=2) as se,
            tc.tile_pool(name="so", bufs=4) as so,
            tc.tile_pool(name="sc", bufs=2) as sc,
            tc.tile_pool(name="dram", bufs=1, space="DRAM") as dram,
            tc.tile_pool(name="ps", bufs=1, space="PSUM") as ps,
            tc.tile_pool(name="pu", bufs=1, space="PSUM") as pu,
            tc.tile_pool(name="px", bufs=1, space="PSUM") as px,
        ):
            lam_sb = consts.tile([128, 1], FP)
            nc.sync.dma_start(lam_sb[:], lam[:])
            wq_sb = consts.tile([128, DC, HD], BF)
            wk_sb = consts.tile([128, DC, HD], BF)
            wv_sb = consts.tile([128, DC, HD], BF)
            nc.scalar.dma_start(wk_sb[:], wk.rearrange("p (c m) -> p c m", c=DC))

            # warm the ACT table set (exp/ln) off the critical path
            warm = consts.tile([128, 1], FP)
            nc.scalar.activation(warm[:], lam_sb[:],
                                 mybir.ActivationFunctionType.Exp)

            # warm the collective stack: small aligned AllToAll (1KB/chunk)
            # absorbs the ~40us CC init off the critical path
            cc_warm_in = dram.tile([NCORES, 512], BF)
            cc_warm_out = dram.tile([NCORES, 512], BF)
            warm8 = consts.tile([8, 512], BF)
            nc.vector.memset(warm8[:], 0.0)
            eps_sb = consts.tile([128, 1], FP)
            nc.vector.memset(eps_sb[:], EPS)
            nc.gpsimd.dma_start(cc_warm_in[:], warm8[:])
            if not skip_cc:
                nc.gpsimd.collective_compute(
                    "AllToAll",
                    mybir.AluOpType.bypass,
                    replica_groups=[list(range(NCORES))],
                    ins=[cc_warm_in.opt()],
                    outs=[cc_warm_out.opt()],
                )

            wp_sb = consts.tile([128, H, D], BF)

            qT_b = [consts.tile([128, N], BF, name=f"qT_{b}") for b in range(B)]
            kT_b = [consts.tile([128, N], BF, name=f"kT_{b}") for b in range(B)]
            # v per (batch, key-chunk): [key, head_dim] plus a ones column
            # (col 128) so the PV matmul also accumulates the softmax denom.
            va_b = [consts.tile([128, KB, HD + 1], BF, name=f"va_{b}") for b in range(B)]
            for b in range(B):
                nc.vector.memset(va_b[b][:, :, HD : HD + 1], 1.0)

            # pipelined reshard buffers, one collective per 512-token block:
            # [block, dest-core, feature, 64 tokens]
            a2a_in = dram.tile([NBLK, NCORES, 128, 64], BF)
            a2a_out = dram.tile([NBLK, NCORES, 128, 64], BF)

            xT_view = xT.rearrange("(c p) t -> p c t", p=128)

            xxs = {}

            def load_x(tp, engines):
                """DMA the contraction chunks for 1024-token pair tp as
                half-token tiles, all first halves before second halves so
                the first qk group can start ~4us earlier."""
                xx = [[None, None] for _ in range(DC)]
                for hf in range(2):
                    for c in range(DC):
                        t = xa.tile([128, 512], BF, tag=f"xx{c}h{hf}",
                                    name=f"xx_{tp}_{c}_{hf}")
                        hs = slice(tp * 1024 + hf * 512,
                                   tp * 1024 + (hf + 1) * 512)
                        engines[c % len(engines)].dma_start(
                            t[:], xT_view[:, c, hs])
                        xx[c][hf] = t
                xxs[tp] = xx

            def proj_group(which, b, j, pst):
                """One q/k projection group for half-block j of batch b into
                the [128, 512] PSUM region pst, then copy out feature-major."""
                xx = xxs[2 * b + j // 2]
                bs = slice(j * 512, (j + 1) * 512)
                w_sb = wq_sb if which == "q" else wk_sb
                dst = (qT_b if which == "q" else kT_b)[b]
                for c in range(DC):
                    nc.tensor.matmul(pst, w_sb[:, c, :], xx[c][j % 2][:],
                                     start=(c == 0), stop=(c == DC - 1))
                nc.vector.tensor_copy(dst[:, bs], pst)

            def v_group(b, j, pxt):
                """v for half-block j of batch b, directly token-major."""
                xx = xxs[2 * b + j // 2]
                vv = pxt.rearrange("p (s m) -> p s m", s=4)
                for sub in range(4):
                    ss = slice(sub * 128, (sub + 1) * 128)
                    for c in range(DC):
                        nc.tensor.matmul(
                            vv[:, sub, :], xx[c][j % 2][:, ss],
                            wv_sb[:, c, :],
                            start=(c == 0 and sub == 0),
                            stop=(c == DC - 1),
                        )
                kb0 = j * 4
                nc.vector.tensor_copy(va_b[b][:, kb0 : kb0 + 4, 0:HD], vv[:])

            def make_qk_filler(which, b, j):
                def f():
                    pst = px.tile([128, 512], FP, tag="px",
                                  name=f"px_{which}_{b}_{j}")
                    proj_group(which, b, j, pst[:])
                return f

            def make_v_filler(b, j):
                def f():
                    pxt = px.tile([128, 512], FP, tag="px", name=f"px_v_{b}_{j}")
                    v_group(b, j, pxt[:])
                return f

            def score_exp(b, qb, kb):
                """scores + exp for one 128-key chunk of block (b, qb)."""
                qs = slice(qb * 512, (qb + 1) * 512)
                ks = slice(kb * 128, (kb + 1) * 128)
                s12 = ps.tile([128, 1024], FP, tag="s12", bufs=2)
                nc.tensor.matmul(s12[:, 0:512], kT_b[b][0:64, ks],
                                 qT_b[b][0:64, qs], start=True, stop=True)
                nc.tensor.matmul(s12[:, 512:1024], kT_b[b][64:128, ks],
                                 qT_b[b][64:128, qs], start=True, stop=True)
                p12 = pp.tile([128, 1024], BF, tag="p12")
                nc.scalar.activation(p12[:], s12[:],
                                     mybir.ActivationFunctionType.Exp)
                return p12

            def pv(b, p12, U, kb):
                vak = va_b[b][:, kb, :]
                for s in range(2):
                    for sub in range(4):
                        r = UREG[s * 4 + sub]
                        nc.tensor.matmul(
                            U[:, r : r + HD + 1],
                            p12[:, s * 512 + sub * 128 : s * 512 + (sub + 1) * 128],
                            vak,
                            start=(kb == 0 and (s, sub) in UFIRST),
                            stop=(kb == KB - 1),
                        )

            def epilogue_part1(tb, U):
                """Differential combine without softmax division: od =
                d2*U1 - lam*d1*U2 = d1*d2*(a1 - lam*a2); RMSNorm is scale-
                invariant so the 1/(d1*d2) factor drops out (eps folded as a
                negligible Ln bias). Fused sum-of-squares via ttr."""
                Usb = se.tile([128, 1536], FP, tag="usb", name=f"usb_{tb}")
                usb_tiles[tb] = Usb
                nc.vector.tensor_copy(Usb[:], U[:])
                ssum4 = se.tile([128, 4], FP, tag="ms")
                od_t = [None] * 4
                for sub in range(4):
                    r1 = UREG[sub]
                    r2 = UREG[4 + sub]
                    d1 = Usb[:, r1 + HD : r1 + HD + 1]
                    d2 = Usb[:, r2 + HD : r2 + HD + 1]
                    w2 = se.tile([128, 1], FP, tag=f"w2_{sub}")
                    nc.vector.tensor_mul(w2[:], d1, lam_sb[:])
                    t2 = se.tile([128, 128], FP, tag=f"t2_{sub}")
                    nc.vector.tensor_scalar_mul(t2[:], Usb[:, r2 : r2 + HD], w2[:])
                    od = se.tile([128, 128], FP, tag=f"od_{sub}")
                    nc.vector.scalar_tensor_tensor(
                        od[:], Usb[:, r1 : r1 + HD], d2, t2[:],
                        mybir.AluOpType.mult, mybir.AluOpType.subtract,
                    )
                    od_t[sub] = od
                    sq = se.tile([128, 128], FP, tag=f"sq_{sub}")
                    nc.vector.scalar_tensor_tensor(
                        sq[:], od[:], 1.0, od[:],
                        mybir.AluOpType.mult, mybir.AluOpType.mult,
                        accum_out=ssum4[:, sub : sub + 1],
                    )
                return (tb, od_t, ssum4)

            def epilogue_part2(tb, od_t, ssum4):
                """RMSNorm scale (Ln/Exp share the attention exps' ACT table
                set), transpose, stage, fire this block's collective. The
                transposes/staging ride the GpSimd queue so a stall never
                blocks PE- or ACT-feeding DMAs. Deferred into the next
                block's chunk loop so the ACT queue never stalls."""
                rt = se.tile([128, 4], FP, tag="rt")
                nc.scalar.activation(rt[:], ssum4[:],
                                     mybir.ActivationFunctionType.Ln,
                                     scale=1.0 / HD, bias=eps_sb[:])
                rs = se.tile([128, 4], FP, tag="rs")
                nc.scalar.activation(rs[:], rt[:],
                                     mybir.ActivationFunctionType.Exp, scale=-0.5)
                onT4 = so.tile([128, 4, 128], BF, tag="onT4")
                for sub in range(4):
                    on = se.tile([128, 128], BF, tag=f"on_{sub}")
                    nc.vector.tensor_scalar_mul(
                        on[:], od_t[sub][:], rs[:, sub : sub + 1]
                    )
                    nc.sync.dma_start_transpose(onT4[:, sub, :], on[:])
                for sub in range(4):
                    nc.sync.dma_start(
                        a2a_in[tb, 2 * sub : 2 * sub + 2].rearrange(
                            "c p t -> p c t"),
                        onT4[:, sub, :].rearrange("p (c t) -> p c t", c=2),
                    )
                if skip_cc:
                    nc.gpsimd.dma_start(a2a_out[tb], a2a_in[tb])
                else:
                    nc.gpsimd.collective_compute(
                        "AllToAll",
                        mybir.AluOpType.bypass,
                        replica_groups=[list(range(NCORES))],
                        ins=[a2a_in[tb].opt()],
                        outs=[a2a_out[tb].opt()],
                    )

            def epilogue_tail(tb, U):
                """Last block's epilogue: per-sub pipelined DVE->ACT->DMA
                chain reading U PSUM directly (no Usb copy - U is not reused),
                transposes on the now-idle ACT queue, stores on sync, so the
                final A2A fires ~5us after the last PV matmul."""
                onT4 = so.tile([128, 4, 128], BF, tag="onT4")
                for sub in range(4):
                    r1 = UREG[sub]
                    r2 = UREG[4 + sub]
                    d1 = U[:, r1 + HD : r1 + HD + 1]
                    d2 = U[:, r2 + HD : r2 + HD + 1]
                    w2 = se.tile([128, 1], FP, tag=f"w2_{sub}")
                    nc.vector.tensor_mul(w2[:], d1, lam_sb[:])
                    t2 = se.tile([128, 128], FP, tag=f"t2_{sub}")
                    nc.vector.tensor_scalar_mul(t2[:], U[:, r2 : r2 + HD], w2[:])
                    od = se.tile([128, 128], FP, tag=f"od_{sub}")
                    nc.vector.scalar_tensor_tensor(
                        od[:], U[:, r1 : r1 + HD], d2, t2[:],
                        mybir.AluOpType.mult, mybir.AluOpType.subtract,
                    )
                    sq = se.tile([128, 128], FP, tag=f"sq_{sub}")
                    ss1 = se.tile([128, 1], FP, tag=f"ss_{sub}")
                    nc.vector.scalar_tensor_tensor(
                        sq[:], od[:], 1.0, od[:],
                        mybir.AluOpType.mult, mybir.AluOpType.mult,
                        accum_out=ss1[:],
                    )
                    rt1 = se.tile([128, 1], FP, tag=f"rt_{sub}")
                    nc.scalar.activation(rt1[:], ss1[:],
                                         mybir.ActivationFunctionType.Ln,
                                         scale=1.0 / HD, bias=eps_sb[:])
                    rs1 = se.tile([128, 1], FP, tag=f"rs_{sub}")
                    nc.scalar.activation(rs1[:], rt1[:],
                                         mybir.ActivationFunctionType.Exp,
                                         scale=-0.5)
                    on = se.tile([128, 128], BF, tag=f"on_{sub}")
                    nc.vector.tensor_scalar_mul(on[:], od[:], rs1[:])
                    nc.scalar.dma_start_transpose(onT4[:, sub, :], on[:])
                    nc.sync.dma_start(
                        a2a_in[tb, 2 * sub : 2 * sub + 2].rearrange(
                            "c p t -> p c t"),
                        onT4[:, sub, :].rearrange("p (c t) -> p c t", c=2),
                    )
                nc.gpsimd.collective_compute(
                    "AllToAll",
                    mybir.AluOpType.bypass,
                    replica_groups=[list(range(NCORES))],
                    ins=[a2a_in[tb].opt()],
                    outs=[a2a_out[tb].opt()],
                )

            aa_tiles = {}
            aa_loaded = set()
            usb_tiles = {}

            def load_aa(pair, tix):
                """Gather all 8 heads of block 2*pair+tix (one 3D-AP DMA)."""
                if pair not in aa_tiles:
                    aa_tiles[pair] = sc.tile([128, H, 2, 64], BF, tag="aa",
                                             name=f"aa_{pair}")
                nc.gpsimd.dma_start(
                    aa_tiles[pair][:, :, tix],
                    a2a_out[2 * pair + tix].rearrange("h p t -> p h t"),
                )
                aa_loaded.add((pair, tix))

            def cproj_pass(pair, pi):
                """Output projection for block pair (2*pair, 2*pair+1): this
                core's 128 tokens; column pass pi covers output chunks
                oc = 4*pi .. 4*pi+3 (one PSUM bank)."""
                for tix in range(2):
                    if (pair, tix) not in aa_loaded:
                        load_aa(pair, tix)
                aa = aa_tiles[pair]
                ypst = px.tile([128, 512], FP, tag="px", name=f"yps_{pair}_{pi}")
                yps = ypst[:].rearrange("p (o t) -> p o t", o=4)
                for oci in range(4):
                    oc = 4 * pi + oci
                    for hh in range(H):
                        nc.tensor.matmul(
                            yps[:, oci, :],
                            wp_sb[:, hh, oc * 128 : (oc + 1) * 128],
                            aa[:, hh].rearrange("p b t -> p (b t)"),
                            start=(hh == 0),
                            stop=(hh == H - 1),
                        )
                yo = sc.tile([128, 512], FP, tag="yo", name=f"yo_{pair}_{pi}")
                nc.vector.tensor_copy(yo[:], ypst[:])
                nc.gpsimd.dma_start(
                    yT.rearrange("(oc p) t -> p oc t", p=128)[
                        :, 4 * pi : 4 * pi + 4, pair * 128 : (pair + 1) * 128
                    ],
                    yo[:].rearrange("p (o t) -> p o t", o=4),
                )

            def b_block(b, qb, tb, fillers, pending, last=False):
                """One 512-token attention block, software-pipelined: scores
                and exp lead the PV accumulation by one chunk; the previous
                block's deferred epilogue tail lands at kb==5; `fillers`
                maps kb -> callable (qkv groups or output-proj passes)."""
                U = pu.tile([128, 1536], FP, tag="U", name=f"U_{tb}")
                p12_prev = None
                for kb in range(KB + 1):
                    p12 = score_exp(b, qb, kb) if kb < KB else None
                    if kb >= 1:
                        pv(b, p12_prev, U[:], kb - 1)
                    if p12 is not None:
                        p12_prev = p12
                    if kb == 5 and pending is not None:
                        epilogue_part2(*pending)
                        pending = None
                    f = fillers.get(kb)
                    if f is not None:
                        f()
                if last:
                    return ("tail", tb, U)
                return epilogue_part1(tb, U[:])

            # ============ startup: x DMA + batch-0 k, then q/v of hb 0 =====
            load_x(0, [nc.sync, nc.scalar])
            nc.sync.dma_start(wq_sb[:], wq.rearrange("p (c m) -> p c m", c=DC))
            load_x(1, [nc.sync, nc.scalar])
            nc.scalar.dma_start(wv_sb[:], wv.rearrange("p (c m) -> p c m", c=DC))
            nc.scalar.dma_start(wp_sb[:], wp.rearrange("p (h m) -> p h m", h=H))

            sA = ps.tile([128, 1024], FP, tag="s12", bufs=2, name="ph1_kA")
            proj_group("k", 0, 0, sA[:, 0:512])
            proj_group("k", 0, 1, sA[:, 512:1024])
            sB = ps.tile([128, 1024], FP, tag="s12", bufs=2, name="ph1_kB")
            proj_group("k", 0, 2, sB[:, 0:512])
            proj_group("k", 0, 3, sB[:, 512:1024])
            pq0 = px.tile([128, 512], FP, tag="px", name="px_q00")
            proj_group("q", 0, 0, pq0[:])
            pv0 = px.tile([128, 512], FP, tag="px", name="px_v00")
            v_group(0, 0, pv0[:])

            # ============ the 8-block attention loop =====================
            def seq(*fs):
                def g():
                    for f in fs:
                        f()
                return g

            def make_aa_filler(pair):
                return lambda: (load_aa(pair, 0), load_aa(pair, 1))

            filler_map = {
                0: {2: make_v_filler(0, 1), 5: make_v_filler(0, 2),
                    8: make_v_filler(0, 3), 11: make_qk_filler("q", 0, 1)},
                1: {2: make_qk_filler("q", 0, 2), 7: make_qk_filler("q", 0, 3),
                    12: make_qk_filler("k", 1, 0)},
                2: {2: make_qk_filler("k", 1, 1), 7: make_qk_filler("k", 1, 2),
                    12: make_qk_filler("k", 1, 3)},
                3: {2: make_qk_filler("q", 1, 0), 7: make_v_filler(1, 0),
                    12: make_v_filler(1, 1)},
                4: {2: make_v_filler(1, 2), 6: make_v_filler(1, 3),
                    10: make_qk_filler("q", 1, 1)},
                5: {2: seq(make_aa_filler(0), make_qk_filler("q", 1, 2)),
                    6: (lambda: cproj_pass(0, 0)),
                    13: (lambda: cproj_pass(0, 1))},
                6: {2: seq(make_aa_filler(1), make_qk_filler("q", 1, 3)),
                    7: (lambda: cproj_pass(1, 0)),
                    13: (lambda: cproj_pass(1, 1))},
                7: {3: make_aa_filler(2),
                    8: (lambda: cproj_pass(2, 0)),
                    13: (lambda: cproj_pass(2, 1))},
            }

            pending = None
            for tb in range(NBLK):
                if tb == 1:
                    load_x(2, [nc.sync])
                elif tb == 2:
                    load_x(3, [nc.sync])
                b, qb = tb // QB, tb % QB
                pending = b_block(b, qb, tb, filler_map.get(tb, {}), pending,
                                  last=(tb == NBLK - 1))

            # ============ tail: last block's pipelined epilogue + pair 3 ==
            _, tb7, U7 = pending
            epilogue_tail(tb7, U7[:])
            load_aa(3, 0)
            cproj_pass(3, 0)
            cproj_pass(3, 1)

            if dbg:
                # round-trip through SBUF at the very end so the dump
                # reflects post-collective DRAM state
                for tb in range(NBLK):
                    for hh in range(NCORES):
                        dt_ = sc.tile([128, 64], BF, tag="dbg")
                        nc.sync.dma_start(dt_[:], a2a_in[tb, hh])
                        nc.sync.dma_start(d_in[tb, hh], dt_[:])
                        dt2 = sc.tile([128, 64], BF, tag="dbg2")
                        nc.sync.dma_start(dt2[:], a2a_out[tb, hh])
                        nc.sync.dma_start(d_out[tb, hh], dt2[:])
                for b in range(B):
                    dq = sc.tile([128, N], BF, tag="dbgq")
                    nc.sync.dma_start(dq[:], qT_b[b][:])
                    nc.sync.dma_start(d_q[b], dq[:])
                    dk = sc.tile([128, N], BF, tag="dbgk")
                    nc.sync.dma_start(dk[:], kT_b[b][:])
                    nc.sync.dma_start(d_k[b], dk[:])
                    dv = sc.tile([128, KB * (HD + 1)], BF, tag="dbgv")
                    nc.sync.dma_start(
                        dv[:], va_b[b][:].rearrange("p a c -> p (a c)"))
                    nc.sync.dma_start(d_v[b], dv[:])

    _split_all_multiwaits(nc)
    return nc


_PROGRAM = None


def _get_program():
    global _PROGRAM
    if _PROGRAM is None:
        _PROGRAM = build_program()
    return _PROGRAM


# ---------------------------------------------------------------- host side
def _prep_in_maps(x, w_qkv, w_proj, lambda_q1, lambda_k1, lambda_q2, lambda_k2,
                  rms_weight):
    import ml_dtypes

    x = np.asarray(x, dtype=np.float32)
    w_qkv = np.asarray(w_qkv, dtype=np.float32)
    w_proj = np.asarray(w_proj, dtype=np.float32)
    xT = np.ascontiguousarray(x.reshape(T, D).T).astype(ml_dtypes.bfloat16)
    lam_val = (
        float(np.exp(np.sum(np.asarray(lambda_q1, np.float64) * np.asarray(lambda_k1, np.float64))))
        - float(np.exp(np.sum(np.asarray(lambda_q2, np.float64) * np.asarray(lambda_k2, np.float64))))
        + LAMBDA_INIT
    )
    lam_arr = np.full((128, 1), lam_val, dtype=np.float32)
    # fold rms_weight and (1 - lambda_init) into the output projection rows
    rw = np.asarray(rms_weight, np.float32)
    wp_full = np.ascontiguousarray(
        w_proj * np.tile(rw, H)[:, None] * np.float32(1.0 - LAMBDA_INIT)
    )

    def chunked(w):  # [D, HD] -> [128, DC*HD] with [p, c*HD+m] = w[c*128+p, m]
        dc = D // 128
        return np.ascontiguousarray(
            w.reshape(dc, 128, HD).transpose(1, 0, 2).reshape(128, dc * HD)
        ).astype(ml_dtypes.bfloat16)

    wp_dev = np.ascontiguousarray(
        wp_full.reshape(H, 128, D).transpose(1, 0, 2).reshape(128, H * D)
    ).astype(ml_dtypes.bfloat16)
    in_maps = []
    for h in range(NCORES):
        hs = slice(h * HD, (h + 1) * HD)
        in_maps.append(
            {
                "xT": xT,
                "wq": chunked(np.ascontiguousarray(w_qkv[:, hs]) * np.float32(SCALE)),
                "wk": chunked(w_qkv[:, PROJ + h * HD : PROJ + (h + 1) * HD]),
                "wv": chunked(w_qkv[:, 2 * PROJ + h * HD : 2 * PROJ + (h + 1) * HD]),
                "wp": wp_dev,
                "lam": lam_arr,
            }
        )
    return in_maps


def _assemble(results):
    y = np.empty((T, D), dtype=np.float32)
    for c in range(NCORES):
        yTc = results[c]["yT"]  # [D, 512], cols ordered (tb, i)
        for tb in range(NBLK):
            y[tb * 512 + c * 64 : tb * 512 + (c + 1) * 64, :] = (
                yTc[:, tb * 64 : (tb + 1) * 64].T
            )
    return y.reshape(B, N, D)


def kernel(x, w_qkv, w_proj, lambda_q1, lambda_k1, lambda_q2, lambda_k2,
           rms_weight):
    nc = _get_program()
    in_maps = _prep_in_maps(
        x, w_qkv, w_proj, lambda_q1, lambda_k1, lambda_q2, lambda_k2, rms_weight
    )
    res = run_bass_kernel_spmd(nc, in_maps, list(range(NCORES)))
    return _assemble(res.results)
